# revision 17
# baseline (speedup 1.0000x reference)
"""Distributed 2-layer GraphSAGE encoder (mean aggregation) on 8 TRN2 NeuronCores.

Strategy (dst-sharded, quota + overflow):
  - Core c owns destination nodes [c*S, (c+1)*S) and every edge whose dst
    lands there.  Source features are read from a full replicated table in
    device DRAM via SWDGE indexed DMA (dma_gather): gathered rows land
    [dst-slot(partition), q(col-block), feat] -- stream position
    j = q*128 + slot inside each per-(superblock, range) sub-block.
  - Main pass: every dst gets a fixed quota of Q slots per source range
    (4 ranges of 25000 nodes; int16 gather indices limit a call's table
    slice to <=32768 rows).  One 4-axis DVE tensor_reduce per superblock
    segment-sums all 4 ranges' [slot, q, feat] rectangles at once.  Quota
    shortfall is padded with indices pointing at spare all-zero table rows.
  - Overflow pass: edges beyond the quota (~10%) go through 128-edge
    one-hot TensorEngine matmuls (rhs = gathered messages, lhsT = one-hot
    dst-slot matrix built on-chip by DVE is_equal against an iota tile),
    accumulating [slot, feat] in PSUM per superblock.
  - Self terms use SBUF-resident transposed features (x for layer 1, h1
    for layer 2) via one matmul per superblock; layer-2's gather table
    rows are y = h1 @ W2l.T built during layer-1 finalize and AllGathered.
  - The table row map row(n) = n + n//12500 leaves one spare zero row per
    12500-row core shard, so the AllGather (plain concat of [12501, 128]
    shards) lands every range slice with an addressable zero row.
  - Both layers share identical index streams (same graph, same row map).
"""

import numpy as np

_REPO = "/opt/trn_rl_repo"

P = 128            # SBUF partitions
Q = 5              # main-pass slots per (dst, range)
NR = 4             # source ranges (int16 index limit)
RNG = 25000        # nodes per range
SH = 12500         # nodes per core shard
SHR = 12501        # table rows per core shard (incl. spare zero row)
SBN = 128          # dst nodes per superblock
B = 4              # superblocks per batch (one gather call group)
PADIDX = 12500     # range-local index of the first spare zero row in a range
SENT = 255.0       # overflow dst-slot sentinel (never matches iota 0..127)


def _ceil(a, b):
    return -(-a // b)


def _host_prep(edge_index, n_cores):
    N = n_cores * SH
    src = edge_index[0].astype(np.int64)
    dst = edge_index[1].astype(np.int64)
    deg = np.bincount(dst, minlength=N).astype(np.float32)
    invdeg = (1.0 / np.maximum(deg, 1.0)).astype(np.float32)

    NSB = _ceil(SH, SBN)
    NBATCH = _ceil(NSB, B)

    r_all = src // RNG
    nl = src - r_all * RNG
    li_all = nl + (nl >= SH)

    per_core = []
    ovf_cnt = np.zeros((n_cores, NSB, NR), np.int64)
    for c in range(n_cores):
        m = (dst >= c * SH) & (dst < (c + 1) * SH)
        es_li, es_r, ed = li_all[m], r_all[m], dst[m] - c * SH
        order = np.lexsort((es_li, es_r, ed))
        es_li, es_r, ed = es_li[order], es_r[order], ed[order]
        gid = ed * NR + es_r
        grp_start = np.searchsorted(gid, gid)
        rank = np.arange(len(gid)) - grp_start
        main = rank < Q
        sb = ed >> 7
        np.add.at(ovf_cnt[c], (sb[~main], es_r[~main]), 1)
        per_core.append((es_li, es_r, ed, rank, main))
    nch_sb = _ceil(ovf_cnt.max(axis=0), 128)          # [NSB, NR] chunks per sb

    MAINC = B * SBN * Q
    batches = []
    idx_off = 0
    ch_off = 0
    for nb in range(NBATCH):
        sbs = list(range(nb * B, min((nb + 1) * B, NSB)))
        calls = []
        for r in range(NR):
            calls.append(dict(kind="main", r=r, nidx=MAINC, idx_lo=idx_off))
            idx_off += MAINC
        for r in range(NR):
            k = int(sum(nch_sb[s, r] for s in sbs))
            if k == 0:
                continue
            chunk_sb = []
            for s in sbs:
                chunk_sb += [s - nb * B] * int(nch_sb[s, r])
            calls.append(dict(kind="ovf", r=r, nidx=k * 128, idx_lo=idx_off,
                              nch=k, ch_lo=ch_off, chunk_sb=chunk_sb))
            idx_off += k * 128
            ch_off += k
        seen = {}
        for call in calls:
            if call["kind"] != "ovf":
                continue
            for j, sl in enumerate(call["chunk_sb"]):
                seen.setdefault(sl, []).append((call, j))
        chunk_marks = {}
        for sl, lst in seen.items():
            for i, (call, j) in enumerate(lst):
                chunk_marks[(id(call), j)] = (sl, i == 0, i == len(lst) - 1)
        batches.append(dict(nb=nb, sbs=sbs, calls=calls, marks=chunk_marks,
                            ovf_sbs=sorted(seen.keys())))
    TOTIDX = idx_off
    TOTCH = max(ch_off, 1)

    chunk_base = np.zeros((NSB, NR), np.int64)
    call_of = {}
    for bt in batches:
        for call in bt["calls"]:
            if call["kind"] != "ovf":
                continue
            run = 0
            for s in bt["sbs"]:
                chunk_base[s, call["r"]] = run
                run += int(nch_sb[s, call["r"]])
            call_of[(bt["nb"], call["r"])] = call

    call_idx_lo = np.zeros((NBATCH, NR), np.int64)
    for bt in batches:
        for call in bt["calls"]:
            if call["kind"] == "main":
                call_idx_lo[bt["nb"], call["r"]] = call["idx_lo"]
    callrows = np.array([[call_of[(b_, r_)]["idx_lo"] if (b_, r_) in call_of else -1
                          for r_ in range(NR)] for b_ in range(NBATCH)])
    chrows = np.array([[call_of[(b_, r_)]["ch_lo"] if (b_, r_) in call_of else -1
                        for r_ in range(NR)] for b_ in range(NBATCH)])

    idx_dev_all, loc_dev_all, invdeg_dev_all = [], [], []
    for c in range(n_cores):
        es_li, es_r, ed, rank, main = per_core[c]
        sb = ed >> 7
        slot = ed & 127
        nb = sb // B
        sbl = sb - nb * B

        idx_flat = np.full(TOTIDX, PADIDX, np.int16)
        loc_flat = np.full(TOTCH * 128, SENT, np.float16)

        # main edges: position in call = sbl*(128*Q) + rank*128 + slot
        # (q-major so non-transpose gather puts slot on the partition axis)
        mm = main
        pos_m = (call_idx_lo[nb[mm], es_r[mm]]
                 + sbl[mm] * (SBN * Q) + rank[mm] * SBN + slot[mm])
        idx_flat[pos_m] = es_li[mm].astype(np.int16)

        # overflow edges: rank within the (sb, range) group
        om = ~main
        osb, orr, osl, onb = sb[om], es_r[om], slot[om], nb[om]
        oli = es_li[om]
        okey = osb * NR + orr
        oord = np.argsort(okey, kind="stable")
        osb, orr, osl, onb, oli = (a[oord] for a in (osb, orr, osl, onb, oli))
        ok_s = okey[oord]
        ovfrank = np.arange(len(ok_s)) - np.searchsorted(ok_s, ok_s)
        chunk_in_sb = ovfrank >> 7
        pos_in_chunk = ovfrank & 127
        ci = chunk_base[osb, orr] + chunk_in_sb
        pos_o = callrows[onb, orr] + ci * 128 + pos_in_chunk
        idx_flat[pos_o] = oli.astype(np.int16)
        gch = chrows[onb, orr] + ci
        loc_flat[gch * 128 + pos_in_chunk] = osl.astype(np.float16)

        idx_dev = np.zeros((128, TOTIDX // 16), np.int16)
        for bt in batches:
            for call in bt["calls"]:
                lo, n = call["idx_lo"], call["nidx"]
                seg = idx_flat[lo:lo + n].reshape(-1, 16).T
                idx_dev[:, lo // 16:(lo + n) // 16] = np.tile(seg, (8, 1))
        loc_dev = np.ascontiguousarray(loc_flat.reshape(TOTCH, 128).T)

        iv = np.ones(NSB * SBN, np.float32)
        iv[:SH] = invdeg[c * SH:(c + 1) * SH]
        invdeg_dev = np.ascontiguousarray(iv.reshape(NSB, SBN).T)

        idx_dev_all.append(idx_dev)
        loc_dev_all.append(loc_dev)
        invdeg_dev_all.append(invdeg_dev)

    meta = dict(N=N, NSB=NSB, NBATCH=NBATCH, TOTIDX=TOTIDX, TOTCH=TOTCH,
                batches=batches,
                MAXBCH=int(max((sum(int(nch_sb[s, r]) for s in bt["sbs"]
                                    for r in range(NR)) for bt in batches),
                               default=1)),
                MAXNCH=int(max((sum(int(nch_sb[s, r]) for s in bt["sbs"])
                                for bt in batches for r in range(NR)), default=1)))
    return meta, idx_dev_all, loc_dev_all, invdeg_dev_all


def _build_program(meta, n_cores):
    import concourse.bacc as bacc
    import concourse.mybir as mybir
    import concourse.tile as tile

    dt = mybir.dt
    AF = mybir.ActivationFunctionType
    ALU = mybir.AluOpType
    AX = mybir.AxisListType

    N, NSB, NBATCH = meta["N"], meta["NSB"], meta["NBATCH"]
    TOTIDX, TOTCH = meta["TOTIDX"], meta["TOTCH"]
    MAXBCH, MAXNCH = meta["MAXBCH"], meta["MAXNCH"]
    batches = meta["batches"]
    ICMAX = max((bt["calls"][-1]["idx_lo"] + bt["calls"][-1]["nidx"]
                 - bt["calls"][0]["idx_lo"]) // 16 for bt in batches)
    TROWS = N + n_cores
    MAINC = B * SBN * Q

    nc = bacc.Bacc(num_swdge_queues=4)

    xtab_ext = nc.dram_tensor("xtab", [TROWS, 128], dt.float16, kind="ExternalInput")
    xsh_ext = nc.dram_tensor("xsh", [SH, 128], dt.float16, kind="ExternalInput")
    idx_ext = nc.dram_tensor("idx16", [128, TOTIDX // 16], dt.int16, kind="ExternalInput")
    loc_ext = nc.dram_tensor("ovfloc", [128, TOTCH], dt.float16, kind="ExternalInput")
    invdeg_ext = nc.dram_tensor("invdeg", [128, NSB], dt.float32, kind="ExternalInput")
    ident_ext = nc.dram_tensor("ident", [128, 128], dt.float16, kind="ExternalInput")
    iota_ext = nc.dram_tensor("iota", [128, 128], dt.float16, kind="ExternalInput")
    w1l_ext = nc.dram_tensor("w1l_t", [128, 64], dt.float16, kind="ExternalInput")
    w1r_ext = nc.dram_tensor("w1r_t", [128, 64], dt.float16, kind="ExternalInput")
    w2l_ext = nc.dram_tensor("w2l_t", [64, 64], dt.float16, kind="ExternalInput")
    w2r_ext = nc.dram_tensor("w2r_t", [64, 64], dt.float16, kind="ExternalInput")
    wlin_ext = nc.dram_tensor("wlin_t", [64, 64], dt.float16, kind="ExternalInput")
    b1_ext = nc.dram_tensor("bias1", [128, 64], dt.float32, kind="ExternalInput")
    b2_ext = nc.dram_tensor("bias2", [128, 64], dt.float32, kind="ExternalInput")
    blin_ext = nc.dram_tensor("biaslin", [128, 64], dt.float32, kind="ExternalInput")
    zero_ext = nc.dram_tensor("zrow", [1, 128], dt.float16, kind="ExternalInput")
    out_ext = nc.dram_tensor("out", [SH, 64], dt.float32, kind="ExternalOutput")

    core_ids = list(range(n_cores))

    with tile.TileContext(nc) as tc:
        with (
            tc.tile_pool(name="const", bufs=1) as constp,
            tc.tile_pool(name="res", bufs=1) as resp,
            tc.tile_pool(name="stream", bufs=2) as strp,
            tc.tile_pool(name="msg", bufs=1) as msgp,
            tc.tile_pool(name="acc", bufs=2) as accp,
            tc.tile_pool(name="fin", bufs=3) as finp,
            tc.tile_pool(name="psO", bufs=1, space="PSUM") as psO,
            tc.tile_pool(name="psF", bufs=1, space="PSUM") as psF,
            tc.tile_pool(name="psT", bufs=1, space="PSUM") as psT,
        ):
            ident_t = constp.tile([128, 128], dt.float16, name="ident_t")
            iota_t = constp.tile([128, 128], dt.float16, name="iota_t")
            invdeg_t = constp.tile([128, NSB], dt.float32, name="invdeg_t")
            w1l_t = constp.tile([128, 64], dt.float16, name="w1l_t")
            w1r_t = constp.tile([128, 64], dt.float16, name="w1r_t")
            w2l_t = constp.tile([64, 64], dt.float16, name="w2l_t")
            w2r_t = constp.tile([64, 64], dt.float16, name="w2r_t")
            wlin_t = constp.tile([64, 64], dt.float16, name="wlin_t")
            b1_t = constp.tile([128, 64], dt.float32, name="b1_t")
            b2_t = constp.tile([128, 64], dt.float32, name="b2_t")
            blin_t = constp.tile([128, 64], dt.float32, name="blin_t")
            zero_t = constp.tile([1, 128], dt.float16, name="zero_t")
            for t, e in ((ident_t, ident_ext), (iota_t, iota_ext),
                         (invdeg_t, invdeg_ext),
                         (w1l_t, w1l_ext), (w1r_t, w1r_ext),
                         (w2l_t, w2l_ext), (w2r_t, w2r_ext),
                         (wlin_t, wlin_ext), (b1_t, b1_ext), (b2_t, b2_ext),
                         (blin_t, blin_ext), (zero_t, zero_ext)):
                nc.sync.dma_start(out=t[:, :], in_=e[:, :])

            xT16 = resp.tile([128, NSB * SBN], dt.float16, name="xT16")
            h1T = resp.tile([64, NSB * SBN], dt.float16, name="h1T")

            tab2_sh = nc.dram_tensor("tab2_sh", [SHR, 128], dt.float16)
            tab2 = nc.dram_tensor("tab2", [TROWS, 128], dt.float16,
                                  addr_space="Shared")

            nc.sync.dma_start(out=tab2_sh[SH:SH + 1, :], in_=zero_ext[:, :])

            def rows_of(sb):
                return min(SBN, SH - sb * SBN)

            # ===== prologue: transpose own x shard into resident xT16 =====
            for sb in range(NSB):
                rows = rows_of(sb)
                nsl = slice(sb * SBN, sb * SBN + rows)
                x16_t = finp.tile([128, 128], dt.float16, name="x16_t")
                nc.sync.dma_start(out=x16_t[:rows, :], in_=xsh_ext[nsl, :])
                tp = psT.tile([128, 128], dt.float16, name="tp", tag="tp",
                              bufs=2)
                nc.tensor.transpose(out=tp[:, :rows], in_=x16_t[:rows, :],
                                    identity=ident_t[:rows, :rows])
                nc.scalar.activation(out=xT16[:, nsl], in_=tp[:, :rows],
                                     func=AF.Copy)

            qload = [0, 0, 0, 0]

            def agg_layer(layer, table):
                for bt in batches:
                    nb = bt["nb"]
                    sbs = bt["sbs"]

                    lo = bt["calls"][0]["idx_lo"]
                    hi = bt["calls"][-1]["idx_lo"] + bt["calls"][-1]["nidx"]
                    icols = (hi - lo) // 16
                    idx_t = strp.tile([128, ICMAX], dt.int16, name="idx_t",
                                      tag="idx", bufs=3)
                    nc.sync.dma_start(out=idx_t[:, :icols],
                                      in_=idx_ext[:, lo // 16:hi // 16])

                    # merged main tile: [128, NR, B*Q, 128]
                    mm_t = msgp.tile([128, NR, B * Q, 128], dt.float16,
                                     name="mm_t", tag="mm", bufs=3)
                    ovfs = []
                    for call in bt["calls"]:
                        r = call["r"]
                        tslice = table[r * (RNG + 2):(r + 1) * (RNG + 2), :]
                        clo = (call["idx_lo"] - lo) // 16
                        chi = clo + call["nidx"] // 16
                        qn = min(range(4), key=lambda q: qload[q])
                        qload[qn] += call["nidx"]
                        if call["kind"] == "main":
                            nc.gpsimd.dma_gather(
                                mm_t[:, r, :, :], tslice,
                                idx_t[:, clo:chi],
                                call["nidx"], call["nidx"], 128,
                                single_packet=False,
                                queue_num=qn)
                        else:
                            k = call["nch"]
                            ot = msgp.tile([128, MAXNCH, 128], dt.float16,
                                           name="omt", tag=f"om{r}", bufs=3)
                            ovfs.append((call, ot))
                            nc.gpsimd.dma_gather(
                                ot[:, :k, :], tslice,
                                idx_t[:, clo:chi],
                                call["nidx"], call["nidx"], 128,
                                single_packet=False,
                                queue_num=qn)

                    tot_ch = sum(c_["nch"] for c_, _ in ovfs)
                    oh_t = None
                    ch0 = 0
                    if tot_ch:
                        ch0 = min(c_["ch_lo"] for c_, _ in ovfs)
                        loc_t = strp.tile([128, MAXBCH], dt.float16,
                                          name="loc_t", tag="loc", bufs=3)
                        nc.sync.dma_start(out=loc_t[:, :tot_ch],
                                          in_=loc_ext[:, ch0:ch0 + tot_ch])
                        oh_t = accp.tile([128, MAXBCH, 128], dt.float16,
                                         name="oh_t", tag="oh")
                        nc.vector.tensor_tensor(
                            out=oh_t[:, :tot_ch, :],
                            in0=loc_t[:, :tot_ch].rearrange(
                                "p (c one) -> p c one",
                                one=1).to_broadcast([128, tot_ch, 128]),
                            in1=iota_t[:, :].rearrange(
                                "p (one w) -> p one w",
                                one=1).to_broadcast([128, tot_ch, 128]),
                            op=ALU.is_equal)

                    # group each superblock's chunks consecutively: only one
                    # open PSUM accumulation group per bank at a time
                    po_t = None
                    if tot_ch:
                        po_t = psO.tile([128, B, 128], dt.float32, name="po_t",
                                        tag="po", bufs=2)
                    by_sl = {}
                    for call, ot in ovfs:
                        for j in range(call["nch"]):
                            sl, _, _ = bt["marks"][(id(call), j)]
                            by_sl.setdefault(sl, []).append((call, ot, j))
                    for sl in sorted(by_sl):
                        lst = by_sl[sl]
                        for i, (call, ot, j) in enumerate(lst):
                            cj = call["ch_lo"] - ch0 + j
                            nc.tensor.matmul(
                                out=po_t[:, sl, :],
                                lhsT=oh_t[:, cj, :], rhs=ot[:, j, :],
                                start=(i == 0), stop=(i == len(lst) - 1),
                                skip_group_check=True)

                    for si, s in enumerate(sbs):
                        rows = rows_of(s)
                        nsl = slice(s * SBN, s * SBN + rows)
                        # one 4-axis reduce: [p, f, r, q] over (r, q)
                        view = mm_t[:, :, si * Q:(si + 1) * Q, :].rearrange(
                            "p r q f -> p f r q")
                        rt = accp.tile([128, 128], dt.float32, name="rsum",
                                       tag="rs")
                        nc.vector.tensor_reduce(out=rt[:, :], in_=view,
                                                axis=AX.XY, op=ALU.add)
                        if si in bt["ovf_sbs"]:
                            nc.vector.tensor_tensor(out=rt[:, :],
                                                    in0=rt[:, :],
                                                    in1=po_t[:, si, :],
                                                    op=ALU.add)
                        inv_bc = invdeg_t[:, s:s + 1].to_broadcast([128, 128])
                        agg16 = finp.tile([128, 128], dt.float16,
                                          name="agg16", tag="agg")
                        nc.vector.tensor_tensor(out=agg16[:, :], in0=rt[:, :],
                                                in1=inv_bc, op=ALU.mult)
                        finalize(layer, s, rows, nsl, agg16)

            def finalize(layer, s, rows, nsl, agg16):
                if layer == 1:
                    # aggT = transpose(agg16): [in-feat, dst]
                    tpa = psT.tile([128, 128], dt.float16, name="tpa",
                                   tag="tp", bufs=2)
                    nc.tensor.transpose(out=tpa[:, :rows],
                                        in_=agg16[:rows, :],
                                        identity=ident_t[:rows, :rows])
                    aggT = finp.tile([128, 128], dt.float16, name="aggT",
                                     tag="aggT")
                    nc.scalar.activation(out=aggT[:, :rows],
                                         in_=tpa[:, :rows], func=AF.Copy)
                    hp = psF.tile([128, 64], dt.float32, name="hp", tag="hp",
                                  bufs=2)
                    nc.tensor.matmul(out=hp[:rows, :], lhsT=aggT[:, :rows],
                                     rhs=w1l_t[:, :], start=True, stop=False,
                                     skip_group_check=True)
                    nc.tensor.matmul(out=hp[:rows, :], lhsT=xT16[:, nsl],
                                     rhs=w1r_t[:, :], start=False, stop=True,
                                     skip_group_check=True)
                    nc.vector.tensor_tensor(out=hp[:rows, :],
                                            in0=hp[:rows, :],
                                            in1=b1_t[:rows, :], op=ALU.add)
                    h1row = finp.tile([128, 64], dt.float16, name="h1row",
                                      tag="h1r")
                    nc.scalar.activation(out=h1row[:rows, :],
                                         in_=hp[:rows, :], func=AF.Relu)
                    tph = psT.tile([128, 128], dt.float16, name="tph",
                                   tag="tp", bufs=2)
                    nc.tensor.transpose(out=tph[:64, :rows],
                                        in_=h1row[:rows, :],
                                        identity=ident_t[:rows, :rows])
                    nc.scalar.activation(out=h1T[:, nsl],
                                         in_=tph[:64, :rows], func=AF.Copy)
                    yp = psF.tile([128, 64], dt.float32, name="yp", tag="yp",
                                  bufs=1)
                    nc.tensor.matmul(out=yp[:rows, :], lhsT=h1T[:, nsl],
                                     rhs=w2l_t[:, :], start=True, stop=True,
                                     skip_group_check=True)
                    ytile = finp.tile([128, 128], dt.float16, name="ytile",
                                      tag="yt")
                    nc.vector.memset(ytile[:, 64:], 0.0)
                    nc.scalar.activation(out=ytile[:rows, :64],
                                         in_=yp[:rows, :], func=AF.Copy)
                    nc.sync.dma_start(out=tab2_sh[nsl, :],
                                      in_=ytile[:rows, :])
                else:
                    # h2 = tanh(agg16[:, :64] + h1 @ W2r.T + b2)
                    h2p = psF.tile([128, 64], dt.float32, name="h2p",
                                   tag="hp", bufs=2)
                    nc.tensor.matmul(out=h2p[:rows, :], lhsT=h1T[:, nsl],
                                     rhs=w2r_t[:, :], start=True, stop=True,
                                     skip_group_check=True)
                    nc.vector.tensor_tensor(out=h2p[:rows, :],
                                            in0=h2p[:rows, :],
                                            in1=agg16[:rows, :64],
                                            op=ALU.add)
                    nc.vector.tensor_tensor(out=h2p[:rows, :],
                                            in0=h2p[:rows, :],
                                            in1=b2_t[:rows, :], op=ALU.add)
                    h2row = finp.tile([128, 64], dt.float16, name="h2row",
                                      tag="h1r")
                    nc.scalar.activation(out=h2row[:rows, :],
                                         in_=h2p[:rows, :], func=AF.Tanh)
                    tph = psT.tile([128, 128], dt.float16, name="tph2",
                                   tag="tp", bufs=2)
                    nc.tensor.transpose(out=tph[:64, :rows],
                                        in_=h2row[:rows, :],
                                        identity=ident_t[:rows, :rows])
                    h2t = finp.tile([64, 128], dt.float16, name="h2t",
                                    tag="h2t")
                    nc.scalar.activation(out=h2t[:, :rows],
                                         in_=tph[:64, :rows], func=AF.Copy)
                    lp = psF.tile([128, 64], dt.float32, name="lp", tag="yp",
                                  bufs=1)
                    nc.tensor.matmul(out=lp[:rows, :], lhsT=h2t[:, :rows],
                                     rhs=wlin_t[:, :], start=True, stop=True,
                                     skip_group_check=True)
                    nc.vector.tensor_tensor(out=lp[:rows, :],
                                            in0=lp[:rows, :],
                                            in1=blin_t[:rows, :], op=ALU.add)
                    negmax = finp.tile([128, 1], dt.float32, name="negmax",
                                       tag="nm")
                    nc.vector.tensor_reduce(out=negmax[:rows, :],
                                            in_=lp[:rows, :], axis=AX.X,
                                            op=ALU.max, negate=True)
                    expt = finp.tile([128, 64], dt.float32, name="expt",
                                     tag="ex")
                    nc.scalar.activation(out=expt[:rows, :], in_=lp[:rows, :],
                                         func=AF.Exp, bias=negmax[:rows, :],
                                         scale=1.0)
                    ssum = finp.tile([128, 1], dt.float32, name="ssum",
                                     tag="ss")
                    nc.vector.tensor_reduce(out=ssum[:rows, :],
                                            in_=expt[:rows, :], axis=AX.X,
                                            op=ALU.add)
                    rinv = finp.tile([128, 1], dt.float32, name="rinv",
                                     tag="ri")
                    nc.vector.reciprocal(rinv[:rows, :], ssum[:rows, :])
                    outt = finp.tile([128, 64], dt.float32, name="outt",
                                     tag="ot")
                    rinv_bc = rinv[:rows, :].to_broadcast([rows, 64])
                    nc.vector.tensor_tensor(out=outt[:rows, :],
                                            in0=expt[:rows, :], in1=rinv_bc,
                                            op=ALU.mult)
                    nc.sync.dma_start(out=out_ext[nsl, :], in_=outt[:rows, :])

            agg_layer(1, xtab_ext)
            nc.gpsimd.collective_compute(
                "AllGather", mybir.AluOpType.bypass,
                replica_groups=[core_ids],
                ins=[tab2_sh[:, :]], outs=[tab2[:, :]],
            )
            agg_layer(2, tab2)

    nc.compile()
    return nc


def _run(inputs, trace=False):
    import sys
    if _REPO not in sys.path:
        sys.path.insert(0, _REPO)
    from concourse.bass_utils import run_bass_kernel_spmd

    x = np.asarray(inputs["x"], np.float32)
    edge_index = np.asarray(inputs["edge_index"])
    W1l = np.asarray(inputs["W1l"], np.float32)
    b1l = np.asarray(inputs["b1l"], np.float32)
    W1r = np.asarray(inputs["W1r"], np.float32)
    W2l = np.asarray(inputs["W2l"], np.float32)
    b2l = np.asarray(inputs["b2l"], np.float32)
    W2r = np.asarray(inputs["W2r"], np.float32)
    Wlin = np.asarray(inputs["Wlin"], np.float32)
    blin = np.asarray(inputs["blin"], np.float32)

    n_cores = 8
    N = x.shape[0]

    meta, idx_dev, loc_dev, invdeg_dev = _host_prep(edge_index, n_cores)
    nc = _build_program(meta, n_cores)

    xtab = np.zeros((N + n_cores, 128), np.float16)
    rows = np.arange(N) + np.arange(N) // SH
    xtab[rows] = x.astype(np.float16)

    ident = np.eye(128, dtype=np.float16)
    iota = np.broadcast_to(np.arange(128, dtype=np.float16)[None, :],
                           (128, 128)).copy()
    common = dict(
        ident=np.ascontiguousarray(ident),
        iota=np.ascontiguousarray(iota),
        w1l_t=np.ascontiguousarray(W1l.T.astype(np.float16)),
        w1r_t=np.ascontiguousarray(W1r.T.astype(np.float16)),
        w2l_t=np.ascontiguousarray(W2l.T.astype(np.float16)),
        w2r_t=np.ascontiguousarray(W2r.T.astype(np.float16)),
        wlin_t=np.ascontiguousarray(Wlin.T.astype(np.float16)),
        bias1=np.ascontiguousarray(np.tile(b1l.astype(np.float32), (128, 1))),
        bias2=np.ascontiguousarray(np.tile(b2l.astype(np.float32), (128, 1))),
        biaslin=np.ascontiguousarray(
            np.tile(blin.astype(np.float32), (128, 1))),
        zrow=np.zeros((1, 128), np.float16),
        xtab=xtab,
    )
    in_maps = []
    for c in range(n_cores):
        m = dict(common)
        m["xsh"] = np.ascontiguousarray(xtab[c * SHR:c * SHR + SH, :])
        m["idx16"] = idx_dev[c]
        m["ovfloc"] = loc_dev[c]
        m["invdeg"] = invdeg_dev[c]
        in_maps.append(m)

    res = run_bass_kernel_spmd(nc, in_maps, list(range(n_cores)), trace=trace)
    out = np.concatenate([res.results[c]["out"] for c in range(n_cores)],
                         axis=0)
    return out, res.exec_time_ns


def kernel(**inputs):
    out, _ = _run(inputs, trace=False)
    return out
